# revision 1
# baseline (speedup 1.0000x reference)
"""Trainium2 Bass kernel for the delayed-dense spiking network.

Network (reference semantics):
    s1 = spike(delayed_dense(psp(x),  w1, d1))   # [B, 800, T]
    s3 = spike(delayed_dense(psp(s1), w3, d3))   # [B, 10, T]

psp is a linear causal filter (u[t] = a*u[t-1] + s[t]) and delayed_dense is a
shift-grouped GEMM; psp commutes exactly with the time shifts and (up to fp
rounding ~1e-6 rel) with the channel mixing, so we evaluate each layer as
    spike(psp(sum_s W_s @ shift_s(x)))
which lets both GEMMs run on *binary* activations (exact in bf16).  Delays lie
in [0,4) so only shifts 0..4 are live (5 shift matrices).

Sharding: data-parallel over batch, 8 batch elements per NeuronCore.
Weights/shifted inputs are prepared host-side:
  - w1t: shift-masked transposed weights packed along K: [5*784 -> 3968, 800] bf16
  - w3t: per-shift transposed weights, rows padded 800->896:  [5, 896, 10] bf16
  - xpk: shift-replicated binary input packed along K: [B, 3968, 350] bf16
The layer-1 GEMM contracts over the packed (shift, channel) axis in 31 K-tiles
of 128; layer-2 keeps per-shift K-tiles and realises shifts as column offsets
into a 4-column zero-padded s1 buffer.
"""

import numpy as np
import ml_dtypes

NIN, NHID, NOUT = 784, 800, 10
B, T = 64, 350
NSHIFT = 5            # delays in [0,4) touch integer shifts 0..4
TAU = 10.0
THETA = 10.0
DMAX = 4.0
N_CORES = 8
BPC = B // N_CORES    # batches per core
KP1 = NSHIFT * NIN    # 3920 packed contraction rows, padded to 31*128
K1_TILES = 31         # ceil(3920/128)
K1_PAD = K1_TILES * 128   # 3968
NHID_PAD = 896        # 7*128
M1_TILES = 7          # ceil(800/128)
K2_TILES = 7          # ceil(800/128)
TW = T + 4            # s1 buffer width with 4 leading zero columns

DECAY = float(np.float32(np.exp(np.float64(-1.0 / TAU))))

_BF16 = ml_dtypes.bfloat16


def _masked_shift_weights(w, d):
    """Return list of NSHIFT float32 [O, I] shift matrices (linear interp)."""
    d = np.clip(d.astype(np.float32), 0.0, np.float32(DMAX))
    fl = np.floor(d)
    frac = d - fl
    out = []
    for s in range(NSHIFT):
        ws = w * ((fl == s).astype(np.float32) * (1.0 - frac)
                  + (fl == (s - 1)).astype(np.float32) * frac)
        out.append(ws.astype(np.float32))
    return out


def _prep_host(spike_input, w1, d1, w3, d3):
    """Host-side packing: masked/transposed bf16 weights + packed shifted x."""
    w1s = _masked_shift_weights(w1, d1)           # 5 x [800, 784]
    w1t = np.zeros((K1_PAD, NHID), dtype=_BF16)
    for s in range(NSHIFT):
        w1t[s * NIN:(s + 1) * NIN, :] = w1s[s].T.astype(_BF16)

    w3s = _masked_shift_weights(w3, d3)           # 5 x [10, 800]
    w3t = np.zeros((NSHIFT, NHID_PAD, NOUT), dtype=_BF16)
    for s in range(NSHIFT):
        w3t[s, :NHID, :] = w3s[s].T.astype(_BF16)

    xb = spike_input.astype(_BF16)                # binary -> exact in bf16
    xpk = np.zeros((B, K1_PAD, T), dtype=_BF16)
    for s in range(NSHIFT):
        if s == 0:
            xpk[:, 0:NIN, :] = xb
        else:
            xpk[:, s * NIN:s * NIN + NIN, s:] = xb[:, :, :T - s]
    return xpk, w1t, w3t


def _build_nc(n_batch=BPC):
    import concourse.bacc as bacc
    import concourse.mybir as mybir
    import concourse.tile as tile

    f32 = mybir.dt.float32
    bf16 = mybir.dt.bfloat16
    mult = mybir.AluOpType.mult
    add = mybir.AluOpType.add
    is_ge = mybir.AluOpType.is_ge

    nc = bacc.Bacc(None, target_bir_lowering=False, debug=False)
    xpk_d = nc.dram_tensor("xpk", [n_batch, K1_PAD, T], bf16, kind="ExternalInput")
    w1t_d = nc.dram_tensor("w1t", [K1_PAD, NHID], bf16, kind="ExternalInput")
    w3t_d = nc.dram_tensor("w3t", [NSHIFT, NHID_PAD, NOUT], bf16, kind="ExternalInput")
    out_d = nc.dram_tensor("out", [n_batch, NOUT, T], f32, kind="ExternalOutput")

    with tile.TileContext(nc) as tc:
        with (
            tc.tile_pool(name="const", bufs=1) as constp,
            tc.tile_pool(name="xpool", bufs=2) as xpool,
            tc.tile_pool(name="s1pool", bufs=n_batch) as s1pool,
            tc.tile_pool(name="upool", bufs=3) as upool,
            tc.tile_pool(name="opool", bufs=4) as opool,
            tc.tile_pool(name="psum1", bufs=4, space="PSUM") as psum1,
            tc.tile_pool(name="psum2", bufs=2, space="PSUM") as psum2,
        ):
            w1t = constp.tile([128, K1_TILES, NHID], bf16)
            nc.sync.dma_start(w1t[:], w1t_d.rearrange("(k p) m -> p k m", p=128))
            w3t = constp.tile([128, NSHIFT, K2_TILES, NOUT], bf16)
            nc.sync.dma_start(w3t[:], w3t_d.rearrange("s (k p) o -> p s k o", p=128))
            dec = constp.tile([128, T], f32)
            nc.vector.memset(dec[:], DECAY)

            s1_tiles = []
            for b in range(n_batch):
                xb = xpool.tile([128, K1_TILES, T], bf16)
                nc.sync.dma_start(xb[:], xpk_d[b].rearrange("(k p) c -> p k c", p=128))
                s1b = s1pool.tile([128, K2_TILES, TW], bf16)
                nc.vector.memset(s1b[:], 0.0)
                for m in range(M1_TILES):
                    mw = min(128, NHID - m * 128)
                    ph = psum1.tile([128, T], f32)
                    for k in range(K1_TILES):
                        nc.tensor.matmul(
                            ph[:mw, :],
                            w1t[:, k, m * 128:m * 128 + mw],
                            xb[:, k, :],
                            start=(k == 0),
                            stop=(k == K1_TILES - 1),
                        )
                    u = upool.tile([128, T], f32)
                    nc.vector.tensor_tensor_scan(
                        u[:mw, :], dec[:mw, :], ph[:mw, :], 0.0, mult, add
                    )
                    nc.vector.tensor_scalar(
                        out=s1b[:mw, m, 4:TW], in0=u[:mw, :],
                        scalar1=THETA, scalar2=None, op0=is_ge,
                    )
                s1_tiles.append(s1b)

            for b in range(n_batch):
                s1b = s1_tiles[b]
                p3 = psum2.tile([NOUT, T], f32)
                idx = 0
                for s in range(NSHIFT):
                    for k2 in range(K2_TILES):
                        nc.tensor.matmul(
                            p3[:],
                            w3t[:, s, k2, :],
                            s1b[:, k2, 4 - s:TW - s],
                            start=(idx == 0),
                            stop=(idx == NSHIFT * K2_TILES - 1),
                        )
                        idx += 1
                u3 = opool.tile([NOUT, T], f32)
                nc.vector.tensor_tensor_scan(
                    u3[:], dec[:NOUT, :], p3[:], 0.0, mult, add
                )
                o3 = opool.tile([NOUT, T], f32)
                nc.vector.tensor_scalar(
                    out=o3[:], in0=u3[:], scalar1=THETA, scalar2=None, op0=is_ge,
                )
                nc.sync.dma_start(out_d[b], o3[:])

    nc.compile()
    return nc


def make_in_maps(spike_input, w1, d1, w3, d3):
    xpk, w1t, w3t = _prep_host(spike_input, w1, d1, w3, d3)
    in_maps = []
    for c in range(N_CORES):
        in_maps.append({
            "xpk": np.ascontiguousarray(xpk[c * BPC:(c + 1) * BPC]),
            "w1t": w1t,
            "w3t": w3t,
        })
    return in_maps


def kernel(spike_input, w1, d1, w3, d3):
    from concourse import bass_utils

    nc = _build_nc()
    in_maps = make_in_maps(spike_input, w1, d1, w3, d3)
    res = bass_utils.run_bass_kernel_spmd(nc, in_maps, core_ids=list(range(N_CORES)))
    out = np.concatenate([res.results[c]["out"] for c in range(N_CORES)], axis=0)
    return out.astype(np.float32)


# revision 23
# speedup vs baseline: 164.8652x; 164.8652x over previous
"""Trainium2 Bass kernel for the delayed-dense spiking network.

Network (reference semantics):
    s1 = spike(delayed_dense(psp(x),  w1, d1))   # [B, 800, T]
    s3 = spike(delayed_dense(psp(s1), w3, d3))   # [B, 10, T]

psp is a linear causal filter (u[t] = a*u[t-1] + s[t]) and delayed_dense is a
shift-grouped GEMM; psp commutes exactly with the time shifts and (up to fp
rounding ~1e-6 rel) with the channel mixing, so we evaluate each layer as
    spike(psp(sum_s W_s @ shift_s(x)))
which lets both GEMMs run on *binary* activations (exact in bf16).  Delays lie
in [0,4) so only shifts 0..4 are live (5 shift matrices).

Sharding: data-parallel over batch, 8 batch elements per NeuronCore.

Layer 1 packs the 5 shifts along the contraction axis (K = 5*784 -> 31 tiles
of 128); the shift-replicated binary input is prepared host-side (xpk).
Layer 2 stacks the 5 shift matrices along the output axis (M = 5*10 = 50) and
runs one matmul per K-tile over the full zero-padded s1 width; the shifted
partials are merged either with 5 small SBUF->SBUF DMAs (partition regroup)
plus 4 vector adds, or - for the last batch, where the chain is the kernel
tail - with 5 tiny float32r selector matmuls that keep the merge on the PE.

Host-side prep:
  - w1t: shift-masked transposed weights packed along K: [3968, 800] bf16
  - w3t: shift-stacked transposed weights: [896, 50] bf16 (col = s*10 + o)
  - xpk: shift-replicated binary input packed along K: [B, 3968, 350] bf16
  - sel: shift-unstack selector, sel[s*10+o, s, o] = 1: [50, 5, 10] f32
"""

import numpy as np
import ml_dtypes

NIN, NHID, NOUT = 784, 800, 10
B, T = 64, 350
NSHIFT = 5            # delays in [0,4) touch integer shifts 0..4
TAU = 10.0
THETA = 10.0
DMAX = 4.0
N_CORES = 8
BPC = B // N_CORES    # batches per core
K1_TILES = 31         # ceil(5*784/128)
K1_PAD = K1_TILES * 128   # 3968
NIN_PAD = 896         # 7*128
NHID_PAD = 896        # 7*128
M1_TILES = 7          # ceil(800/128)
K2_TILES = 7          # ceil(800/128)
TW = T + 4            # buffer width with 4 leading zero columns
M2 = NSHIFT * NOUT    # 50

DECAY = float(np.float32(np.exp(np.float64(-1.0 / TAU))))

_BF16 = ml_dtypes.bfloat16


def _masked_shift_weights(w, d):
    """Return list of NSHIFT float32 [O, I] shift matrices (linear interp)."""
    d = np.clip(d.astype(np.float32), 0.0, np.float32(DMAX))
    fl = np.floor(d)
    frac = d - fl
    out = []
    for s in range(NSHIFT):
        ws = w * ((fl == s).astype(np.float32) * (1.0 - frac)
                  + (fl == (s - 1)).astype(np.float32) * frac)
        out.append(ws.astype(np.float32))
    return out


def _prep_host(spike_input, w1, d1, w3, d3):
    w1s = _masked_shift_weights(w1, d1)           # 5 x [800, 784]
    w1t = np.zeros((K1_PAD, NHID), dtype=_BF16)
    for s in range(NSHIFT):
        w1t[s * NIN:(s + 1) * NIN, :] = w1s[s].T.astype(_BF16)

    w3s = _masked_shift_weights(w3, d3)           # 5 x [10, 800]
    w3t = np.zeros((NHID_PAD, M2), dtype=_BF16)
    for s in range(NSHIFT):
        w3t[:NHID, s * NOUT:(s + 1) * NOUT] = w3s[s].T.astype(_BF16)

    xb = spike_input.astype(_BF16)                # binary -> exact in bf16
    xpk = np.zeros((B, K1_PAD, T), dtype=_BF16)
    for s in range(NSHIFT):
        if s == 0:
            xpk[:, 0:NIN, :] = xb
        else:
            xpk[:, s * NIN:s * NIN + NIN, s:] = xb[:, :, :T - s]

    sel = np.zeros((M2, NSHIFT, NOUT), dtype=np.float32)
    for s in range(NSHIFT):
        for o in range(NOUT):
            sel[s * NOUT + o, s, o] = 1.0
    return xpk, w1t, w3t, sel


def _build_nc(n_batch=BPC, rep=1, b0_chunked=True):
    import contextlib
    import concourse.bacc as bacc
    import concourse.mybir as mybir
    import concourse.tile as tile

    f32 = mybir.dt.float32
    bf16 = mybir.dt.bfloat16

    nc = bacc.Bacc(None, target_bir_lowering=False, debug=False)
    xpk_d = nc.dram_tensor("xpk", [n_batch, K1_PAD, T], bf16, kind="ExternalInput")
    w1t_d = nc.dram_tensor("w1t", [K1_PAD, NHID], bf16, kind="ExternalInput")
    w3t_d = nc.dram_tensor("w3t", [NHID_PAD, M2], bf16, kind="ExternalInput")
    sel_d = nc.dram_tensor("sel", [M2, NSHIFT, NOUT], f32, kind="ExternalInput")
    out_d = nc.dram_tensor("out", [n_batch, NOUT, T], f32, kind="ExternalOutput")

    with tile.TileContext(nc) as tc:
        with (
            tc.tile_pool(name="const", bufs=1) as constp,
            tc.tile_pool(name="xpool", bufs=3) as xpool,
            tc.tile_pool(name="s1pool", bufs=3) as s1pool,
            tc.tile_pool(name="upool", bufs=3) as upool,
            tc.tile_pool(name="qpool", bufs=2) as qpool,
            tc.tile_pool(name="opool", bufs=4) as opool,
            tc.tile_pool(name="psum1", bufs=7, space="PSUM") as psum1,
            tc.tile_pool(name="psum2", bufs=1, space="PSUM") as psum2,
        ):
            w1t = constp.tile([128, K1_TILES, NHID], bf16)
            w1t_src = w1t_d.rearrange("(k p) m -> p k m", p=128)
            w3t = constp.tile([128, K2_TILES, M2], bf16)
            dec = constp.tile([128, T], f32)
            sel_f = constp.tile([M2, NSHIFT, NOUT], f32)
            sel_r = constp.tile([M2, NSHIFT, NOUT], mybir.dt.float32r)

            def _emit_consts():
                nc.scalar.dma_start(w3t[:], w3t_d.rearrange("(k p) c -> p k c", p=128))
                nc.vector.memset(dec[:], DECAY)
                nc.scalar.dma_start(sel_f[:], sel_d[:])
                nc.vector.tensor_copy(sel_r[:], sel_f[:])

            loop_ctx = (
                tc.For_i(0, rep, 1, hint_engines=(mybir.EngineType.PE,))
                if rep > 1 else contextlib.nullcontext()
            )
            with loop_ctx:
                _emit_body(nc, tc, n_batch, xpool, s1pool, upool,
                           qpool, opool, psum1, psum2, xpk_d, out_d, w1t,
                           w1t_src, w3t, dec, sel_r, mybir, load_w1t=True,
                           emit_consts=_emit_consts, b0_chunked=b0_chunked)

    nc.compile()
    return nc


_XB_ENGINES = ["scalar", "sync"]


def _load_xpk(nc, mybir, b, xpool, xpk_d, chunked=False):
    """DMA one batch's packed input; rotate issue engines to avoid queue
    contention with the weight-chunk stream."""
    bf16 = mybir.dt.bfloat16
    xb = xpool.tile([128, K1_TILES, T], bf16, tag="xb", name=f"xb_{b}")
    src = xpk_d[b].rearrange("(k p) c -> p k c", p=128)
    if chunked:
        for k in range(K1_TILES):
            nc.scalar.dma_start(xb[:, k, :], src[:, k, :])
    else:
        eng = getattr(nc, _XB_ENGINES[b % len(_XB_ENGINES)])
        eng.dma_start(xb[:], src[:])
    return xb


def _emit_l2(nc, mybir, b, s1b, psum2, qpool, opool, dec, w3t, sel_r, out_d,
             tail=False, p3=None):
    """Layer 2 for one batch: M-stacked shift GEMM + partial merge + psp."""
    f32 = mybir.dt.float32
    f32r = mybir.dt.float32r
    mult, add = mybir.AluOpType.mult, mybir.AluOpType.add
    if p3 is None:
        p3 = psum2.tile([M2, TW], f32, tag="p3", name=f"p3_{b}")
        for k2 in range(K2_TILES):
            nc.tensor.matmul(
                p3[:], w3t[:, k2, :], s1b[:, k2, :],
                start=(k2 == 0), stop=(k2 == K2_TILES - 1),
            )
    if tail:
        # Keep the merge on the PE: float32r selector matmuls read the
        # shifted partial slices and accumulate h3 directly in PSUM.
        q50r = qpool.tile([M2, TW], f32r, tag="q50r")
        nc.vector.tensor_copy(q50r[:], p3[:])
        h3p = psum2.tile([M2, TW], f32, tag="p3", name=f"h3p_{b}")
        for s in range(NSHIFT):
            nc.tensor.matmul(
                h3p[:NOUT, :T], sel_r[:, s, :], q50r[:, 4 - s:TW - s],
                start=(s == 0), stop=(s == NSHIFT - 1),
            )
        u3 = opool.tile([NOUT, T], f32, tag="u3", name=f"u3_{b}")
        nc.vector.tensor_tensor_scan(
            u3[:], dec[:NOUT, :], h3p[:NOUT, :T], 0.0, mult, add)
    else:
        q50 = qpool.tile([M2, TW], f32, tag="q50")
        nc.vector.tensor_copy(q50[:], p3[:])
        q = qpool.tile([NOUT, NSHIFT, TW], f32, tag="q")
        dma_engines = [nc.scalar, nc.sync, nc.scalar, nc.sync, nc.scalar]
        for s in range(NSHIFT):
            dma_engines[s].dma_start(q[:, s, :], q50[s * NOUT:(s + 1) * NOUT, :])
        acc = opool.tile([NOUT, T], f32, tag="acc")
        nc.vector.tensor_add(acc[:], q[:, 0, 4:TW], q[:, 1, 3:TW - 1])
        nc.vector.tensor_add(acc[:], acc[:], q[:, 2, 2:TW - 2])
        nc.vector.tensor_add(acc[:], acc[:], q[:, 3, 1:TW - 3])
        nc.vector.tensor_add(acc[:], acc[:], q[:, 4, 0:TW - 4])
        u3 = opool.tile([NOUT, T], f32, tag="u3", name=f"u3_{b}")
        nc.vector.tensor_tensor_scan(u3[:], dec[:NOUT, :], acc[:], 0.0, mult, add)
    o3 = opool.tile([NOUT, T], f32, tag="o3", name=f"o3_{b}")
    nc.vector.tensor_scalar(
        out=o3[:], in0=u3[:], scalar1=THETA, scalar2=None,
        op0=mybir.AluOpType.is_ge,
    )
    nc.sync.dma_start(out_d[b], o3[:])


def _emit_body(nc, tc, n_batch, xpool, s1pool, upool, qpool, opool,
               psum1, psum2, xpk_d, out_d, w1t, w1t_src, w3t, dec, sel_r,
               mybir, load_w1t=True, emit_consts=None, b0_chunked=True):
    f32 = mybir.dt.float32
    bf16 = mybir.dt.bfloat16
    mult, add = mybir.AluOpType.mult, mybir.AluOpType.add
    is_ge = mybir.AluOpType.is_ge

    s1_tiles = [None] * n_batch

    # ---- batch 0: k-outer ordering, chunked weight DMAs, so the PE starts
    # as soon as the first K-chunk of weights lands.
    xb0 = xpool.tile([128, K1_TILES, T], mybir.dt.bfloat16, tag="xb", name="xb_0")
    xb0_src = xpk_d[0].rearrange("(k p) c -> p k c", p=128)
    if b0_chunked:
        for k in range(K1_TILES):
            if load_w1t:
                nc.sync.dma_start(w1t[:, k, :], w1t_src[:, k, :])
            nc.scalar.dma_start(xb0[:, k, :], xb0_src[:, k, :])
            if k == 0 and emit_consts is not None:
                emit_consts()
    else:
        if load_w1t:
            nc.sync.dma_start(w1t[:], w1t_src[:])
        nc.scalar.dma_start(xb0[:], xb0_src[:])
        if emit_consts is not None:
            emit_consts()
    s1b0 = s1pool.tile([128, K2_TILES, TW], bf16, tag="s1b")
    nc.vector.memset(s1b0[:], 0.0)
    phs = [psum1.tile([128, T], f32, tag="phs", name=f"ph{m}") for m in range(M1_TILES)]
    for k in range(K1_TILES):
        for m in range(M1_TILES):
            mw = min(128, NHID - m * 128)
            nc.tensor.matmul(
                phs[m][:mw, :], w1t[:, k, m * 128:m * 128 + mw], xb0[:, k, :],
                start=(k == 0), stop=(k == K1_TILES - 1),
            )
    for m in range(M1_TILES):
        mw = min(128, NHID - m * 128)
        u = upool.tile([128, T], f32, tag="u", name=f"u0_{m}")
        nc.vector.tensor_tensor_scan(u[:mw, :], dec[:mw, :], phs[m][:mw, :], 0.0, mult, add)
        nc.vector.tensor_scalar(
            out=s1b0[:mw, m, 4:TW], in0=u[:mw, :],
            scalar1=THETA, scalar2=None, op0=is_ge,
        )
    s1_tiles[0] = s1b0

    # ---- batches 1..n: m-outer ordering (weights resident); layer 2 of the
    # previous batch is emitted mid-batch so it interleaves on the PE.
    for b in range(1, n_batch):
        xb = _load_xpk(nc, mybir, b, xpool, xpk_d)
        s1b = s1pool.tile([128, K2_TILES, TW], bf16, tag="s1b", name=f"s1b_{b}")
        nc.vector.memset(s1b[:], 0.0)
        for m in range(M1_TILES):
            mw = min(128, NHID - m * 128)
            ph = psum1.tile([128, T], f32, tag="phs", name=f"ph_{b}_{m}")
            for k in range(K1_TILES):
                nc.tensor.matmul(
                    ph[:mw, :], w1t[:, k, m * 128:m * 128 + mw], xb[:, k, :],
                    start=(k == 0), stop=(k == K1_TILES - 1),
                )
            u = upool.tile([128, T], f32, tag="u", name=f"u_{b}_{m}")
            nc.vector.tensor_tensor_scan(u[:mw, :], dec[:mw, :], ph[:mw, :], 0.0, mult, add)
            nc.vector.tensor_scalar(
                out=s1b[:mw, m, 4:TW], in0=u[:mw, :],
                scalar1=THETA, scalar2=None, op0=is_ge,
            )
            if m == 1:
                _emit_l2(nc, mybir, b - 1, s1_tiles[b - 1], psum2, qpool,
                         opool, dec, w3t, sel_r, out_d)
        s1_tiles[b] = s1b

    _emit_l2(nc, mybir, n_batch - 1, s1_tiles[n_batch - 1], psum2, qpool,
             opool, dec, w3t, sel_r, out_d, tail=True)


def make_in_maps(spike_input, w1, d1, w3, d3):
    xpk, w1t, w3t, sel = _prep_host(spike_input, w1, d1, w3, d3)
    in_maps = []
    for c in range(N_CORES):
        in_maps.append({
            "xpk": np.ascontiguousarray(xpk[c * BPC:(c + 1) * BPC]),
            "w1t": w1t,
            "w3t": w3t,
            "sel": sel,
        })
    return in_maps


def kernel(spike_input, w1, d1, w3, d3):
    from concourse import bass_utils

    spike_input = np.asarray(spike_input, dtype=np.float32)
    w1 = np.asarray(w1, dtype=np.float32)
    d1 = np.asarray(d1, dtype=np.float32)
    w3 = np.asarray(w3, dtype=np.float32)
    d3 = np.asarray(d3, dtype=np.float32)

    nc = _build_nc()
    in_maps = make_in_maps(spike_input, w1, d1, w3, d3)
    res = bass_utils.run_bass_kernel_spmd(nc, in_maps, core_ids=list(range(N_CORES)))
    out = np.concatenate([res.results[c]["out"] for c in range(N_CORES)], axis=0)
    return out.astype(np.float32)


# revision 26
# speedup vs baseline: 186.7246x; 1.1326x over previous
"""Trainium2 Bass kernel for the delayed-dense spiking network.

Network (reference semantics):
    s1 = spike(delayed_dense(psp(x),  w1, d1))   # [B, 800, T]
    s3 = spike(delayed_dense(psp(s1), w3, d3))   # [B, 10, T]

psp is a linear causal filter (u[t] = a*u[t-1] + s[t]) and delayed_dense is a
shift-grouped GEMM; psp commutes exactly with the time shifts and (up to fp
rounding ~1e-6 rel) with the channel mixing, so we evaluate each layer as
    spike(psp(sum_s W_s @ shift_s(x)))
which lets both GEMMs run on *binary* activations (exact in bf16).  Delays lie
in [0,4) so only shifts 0..4 are live (5 shift matrices).

Sharding: data-parallel over batch, 8 batch elements per NeuronCore.

Layer 1 packs the 5 shifts along the contraction axis (K = 5*784 -> 31 tiles
of 128); the shift-replicated binary input is prepared host-side (xpk).
Layer 2 stacks the 5 shift matrices along the output axis (M = 5*10 = 50) and
runs one matmul per K-tile over the full zero-padded s1 width; the shifted
partials are merged either with 5 small SBUF->SBUF DMAs (partition regroup)
plus 4 vector adds, or - for the last batch, where the chain is the kernel
tail - with 5 tiny float32r selector matmuls that keep the merge on the PE.

Host-side prep:
  - w1t: shift-masked transposed weights packed along K: [3968, 800] bf16
  - w3t: shift-stacked transposed weights: [896, 50] bf16 (col = s*10 + o)
  - xpk: shift-replicated binary input packed along K: [B, 3968, 350] bf16
  - sel: shift-unstack selector, sel[s*10+o, s, o] = 1: [50, 5, 10] f32
"""

import numpy as np
import ml_dtypes

NIN, NHID, NOUT = 784, 800, 10
B, T = 64, 350
NSHIFT = 5            # delays in [0,4) touch integer shifts 0..4
TAU = 10.0
THETA = 10.0
DMAX = 4.0
N_CORES = 8
BPC = B // N_CORES    # batches per core
K1_TILES = 31         # ceil(5*784/128)
K1_PAD = K1_TILES * 128   # 3968
NIN_PAD = 896         # 7*128
NHID_PAD = 896        # 7*128
M1_TILES = 7          # ceil(800/128)
K2_TILES = 7          # ceil(800/128)
TW = T + 4            # buffer width with 4 leading zero columns
M2 = NSHIFT * NOUT    # 50

DECAY = float(np.float32(np.exp(np.float64(-1.0 / TAU))))

_BF16 = ml_dtypes.bfloat16


def _masked_shift_weights(w, d):
    """Return list of NSHIFT float32 [O, I] shift matrices (linear interp)."""
    d = np.clip(d.astype(np.float32), 0.0, np.float32(DMAX))
    fl = np.floor(d)
    frac = d - fl
    out = []
    for s in range(NSHIFT):
        ws = w * ((fl == s).astype(np.float32) * (1.0 - frac)
                  + (fl == (s - 1)).astype(np.float32) * frac)
        out.append(ws.astype(np.float32))
    return out


def _prep_host(spike_input, w1, d1, w3, d3):
    w1s = _masked_shift_weights(w1, d1)           # 5 x [800, 784]
    w1t = np.zeros((K1_PAD, NHID), dtype=_BF16)
    for s in range(NSHIFT):
        w1t[s * NIN:(s + 1) * NIN, :] = w1s[s].T.astype(_BF16)

    w3s = _masked_shift_weights(w3, d3)           # 5 x [10, 800]
    w3t = np.zeros((NHID_PAD, M2), dtype=_BF16)
    for s in range(NSHIFT):
        w3t[:NHID, s * NOUT:(s + 1) * NOUT] = w3s[s].T.astype(_BF16)

    xb = spike_input.astype(_BF16)                # binary -> exact in bf16
    xpk = np.zeros((B, K1_PAD, T), dtype=_BF16)
    for s in range(NSHIFT):
        if s == 0:
            xpk[:, 0:NIN, :] = xb
        else:
            xpk[:, s * NIN:s * NIN + NIN, s:] = xb[:, :, :T - s]

    sel = np.zeros((M2, NSHIFT, NOUT), dtype=np.float32)
    for s in range(NSHIFT):
        for o in range(NOUT):
            sel[s * NOUT + o, s, o] = 1.0
    return xpk, w1t, w3t, sel


def _build_nc(n_batch=BPC, rep=1, b0_chunked=True):
    import contextlib
    import concourse.bacc as bacc
    import concourse.mybir as mybir
    import concourse.tile as tile

    f32 = mybir.dt.float32
    bf16 = mybir.dt.bfloat16

    nc = bacc.Bacc(None, target_bir_lowering=False, debug=False)
    xpk_d = nc.dram_tensor("xpk", [n_batch, K1_PAD, T], bf16, kind="ExternalInput")
    w1t_d = nc.dram_tensor("w1t", [K1_PAD, NHID], bf16, kind="ExternalInput")
    w3t_d = nc.dram_tensor("w3t", [NHID_PAD, M2], bf16, kind="ExternalInput")
    sel_d = nc.dram_tensor("sel", [M2, NSHIFT, NOUT], f32, kind="ExternalInput")
    out_d = nc.dram_tensor("out", [n_batch, NOUT, T], f32, kind="ExternalOutput")

    with tile.TileContext(nc) as tc:
        with (
            tc.tile_pool(name="const", bufs=1) as constp,
            tc.tile_pool(name="xpool", bufs=3) as xpool,
            tc.tile_pool(name="s1pool", bufs=3) as s1pool,
            tc.tile_pool(name="upool", bufs=3) as upool,
            tc.tile_pool(name="qpool", bufs=2) as qpool,
            tc.tile_pool(name="opool", bufs=4) as opool,
            tc.tile_pool(name="psum1", bufs=7, space="PSUM") as psum1,
            tc.tile_pool(name="psum2", bufs=1, space="PSUM") as psum2,
        ):
            w1t = constp.tile([128, K1_TILES, NHID], bf16)
            w1t_src = w1t_d.rearrange("(k p) m -> p k m", p=128)
            w3t = constp.tile([128, K2_TILES, M2], bf16)
            dec = constp.tile([128, T], f32)
            sel_f = constp.tile([M2, NSHIFT, NOUT], f32)
            sel_r = constp.tile([M2, NSHIFT, NOUT], mybir.dt.float32r)

            def _emit_consts():
                nc.scalar.dma_start(w3t[:], w3t_d.rearrange("(k p) c -> p k c", p=128))
                nc.vector.memset(dec[:], DECAY)
                nc.scalar.dma_start(sel_f[:], sel_d[:])
                nc.vector.tensor_copy(sel_r[:], sel_f[:])

            loop_ctx = (
                tc.For_i(0, rep, 1, hint_engines=(mybir.EngineType.PE,))
                if rep > 1 else contextlib.nullcontext()
            )
            with loop_ctx:
                _emit_body(nc, tc, n_batch, xpool, s1pool, upool,
                           qpool, opool, psum1, psum2, xpk_d, out_d, w1t,
                           w1t_src, w3t, dec, sel_r, mybir, load_w1t=True,
                           emit_consts=_emit_consts, b0_chunked=b0_chunked)

    nc.compile()
    return nc


_XB_ENGINES = ["scalar", "sync"]


def _load_xpk(nc, mybir, b, xpool, xpk_d, chunked=False):
    """DMA one batch's packed input; rotate issue engines to avoid queue
    contention with the weight-chunk stream."""
    bf16 = mybir.dt.bfloat16
    xb = xpool.tile([128, K1_TILES, T], bf16, tag="xb", name=f"xb_{b}")
    src = xpk_d[b].rearrange("(k p) c -> p k c", p=128)
    if chunked:
        for k in range(K1_TILES):
            nc.scalar.dma_start(xb[:, k, :], src[:, k, :])
    else:
        eng = getattr(nc, _XB_ENGINES[b % len(_XB_ENGINES)])
        eng.dma_start(xb[:], src[:])
    return xb


def _emit_l2(nc, mybir, b, s1b, psum2, qpool, opool, dec, w3t, sel_r, out_d,
             tail=False, p3=None):
    """Layer 2 for one batch: M-stacked shift GEMM + partial merge + psp."""
    f32 = mybir.dt.float32
    f32r = mybir.dt.float32r
    mult, add = mybir.AluOpType.mult, mybir.AluOpType.add
    if p3 is None:
        p3 = psum2.tile([M2, TW], f32, tag="p3", name=f"p3_{b}")
        for k2 in range(K2_TILES):
            nc.tensor.matmul(
                p3[:], w3t[:, k2, :], s1b[:, k2, :],
                start=(k2 == 0), stop=(k2 == K2_TILES - 1),
            )
    if tail:
        # Keep the merge on the PE: float32r selector matmuls read the
        # shifted partial slices and accumulate h3 directly in PSUM.
        q50r = qpool.tile([M2, TW], f32r, tag="q50r")
        nc.vector.tensor_copy(q50r[:], p3[:])
        h3p = psum2.tile([M2, TW], f32, tag="p3", name=f"h3p_{b}")
        for s in range(NSHIFT):
            nc.tensor.matmul(
                h3p[:NOUT, :T], sel_r[:, s, :], q50r[:, 4 - s:TW - s],
                start=(s == 0), stop=(s == NSHIFT - 1),
            )
        u3 = opool.tile([NOUT, T], f32, tag="u3", name=f"u3_{b}")
        nc.vector.tensor_tensor_scan(
            u3[:], dec[:NOUT, :], h3p[:NOUT, :T], 0.0, mult, add)
    else:
        q50 = qpool.tile([M2, TW], f32, tag="q50")
        nc.vector.tensor_copy(q50[:], p3[:])
        q = qpool.tile([NOUT, NSHIFT, TW], f32, tag="q")
        dma_engines = [nc.scalar, nc.sync, nc.scalar, nc.sync, nc.scalar]
        for s in range(NSHIFT):
            dma_engines[s].dma_start(q[:, s, :], q50[s * NOUT:(s + 1) * NOUT, :])
        acc = opool.tile([NOUT, T], f32, tag="acc")
        nc.vector.tensor_add(acc[:], q[:, 0, 4:TW], q[:, 1, 3:TW - 1])
        nc.vector.tensor_add(acc[:], acc[:], q[:, 2, 2:TW - 2])
        nc.vector.tensor_add(acc[:], acc[:], q[:, 3, 1:TW - 3])
        nc.vector.tensor_add(acc[:], acc[:], q[:, 4, 0:TW - 4])
        u3 = opool.tile([NOUT, T], f32, tag="u3", name=f"u3_{b}")
        nc.vector.tensor_tensor_scan(u3[:], dec[:NOUT, :], acc[:], 0.0, mult, add)
    o3 = opool.tile([NOUT, T], f32, tag="o3", name=f"o3_{b}")
    nc.vector.tensor_scalar(
        out=o3[:], in0=u3[:], scalar1=THETA, scalar2=None,
        op0=mybir.AluOpType.is_ge,
    )
    nc.sync.dma_start(out_d[b], o3[:])


def _emit_body(nc, tc, n_batch, xpool, s1pool, upool, qpool, opool,
               psum1, psum2, xpk_d, out_d, w1t, w1t_src, w3t, dec, sel_r,
               mybir, load_w1t=True, emit_consts=None, b0_chunked=True):
    f32 = mybir.dt.float32
    bf16 = mybir.dt.bfloat16
    mult, add = mybir.AluOpType.mult, mybir.AluOpType.add
    is_ge = mybir.AluOpType.is_ge

    s1_tiles = [None] * n_batch

    # ---- batch 0: k-outer ordering, chunked weight DMAs, so the PE starts
    # as soon as the first K-chunk of weights lands.
    xb0 = xpool.tile([128, K1_TILES, T], mybir.dt.bfloat16, tag="xb", name="xb_0")
    xb0_src = xpk_d[0].rearrange("(k p) c -> p k c", p=128)
    if b0_chunked:
        for k in range(K1_TILES):
            if load_w1t:
                if k == 0:
                    # first matmul needs only cols 0:128 of chunk 0 - land a
                    # small sliver first so the PE starts ~1us earlier
                    nc.sync.dma_start(w1t[:, 0, 0:128], w1t_src[:, 0, 0:128])
                    nc.sync.dma_start(w1t[:, 0, 128:NHID], w1t_src[:, 0, 128:NHID])
                else:
                    nc.sync.dma_start(w1t[:, k, :], w1t_src[:, k, :])
            nc.scalar.dma_start(xb0[:, k, :], xb0_src[:, k, :])
            if k == 0 and emit_consts is not None:
                emit_consts()
    else:
        if load_w1t:
            nc.sync.dma_start(w1t[:], w1t_src[:])
        nc.scalar.dma_start(xb0[:], xb0_src[:])
        if emit_consts is not None:
            emit_consts()
    s1b0 = s1pool.tile([128, K2_TILES, TW], bf16, tag="s1b")
    nc.vector.memset(s1b0[:], 0.0)
    phs = [psum1.tile([128, T], f32, tag="phs", name=f"ph{m}") for m in range(M1_TILES)]
    for k in range(K1_TILES):
        for m in range(M1_TILES):
            mw = min(128, NHID - m * 128)
            nc.tensor.matmul(
                phs[m][:mw, :], w1t[:, k, m * 128:m * 128 + mw], xb0[:, k, :],
                start=(k == 0), stop=(k == K1_TILES - 1),
            )
    for m in range(M1_TILES):
        mw = min(128, NHID - m * 128)
        u = upool.tile([128, T], f32, tag="u", name=f"u0_{m}")
        nc.vector.tensor_tensor_scan(u[:mw, :], dec[:mw, :], phs[m][:mw, :], 0.0, mult, add)
        nc.vector.tensor_scalar(
            out=s1b0[:mw, m, 4:TW], in0=u[:mw, :],
            scalar1=THETA, scalar2=None, op0=is_ge,
        )
    s1_tiles[0] = s1b0

    # ---- batches 1..n: m-outer ordering (weights resident); layer 2 of the
    # previous batch is emitted mid-batch so it interleaves on the PE.
    for b in range(1, n_batch):
        xb = _load_xpk(nc, mybir, b, xpool, xpk_d)
        s1b = s1pool.tile([128, K2_TILES, TW], bf16, tag="s1b", name=f"s1b_{b}")
        nc.vector.memset(s1b[:], 0.0)
        for m in range(M1_TILES):
            mw = min(128, NHID - m * 128)
            ph = psum1.tile([128, T], f32, tag="phs", name=f"ph_{b}_{m}")
            for k in range(K1_TILES):
                nc.tensor.matmul(
                    ph[:mw, :], w1t[:, k, m * 128:m * 128 + mw], xb[:, k, :],
                    start=(k == 0), stop=(k == K1_TILES - 1),
                )
            u = upool.tile([128, T], f32, tag="u", name=f"u_{b}_{m}")
            nc.vector.tensor_tensor_scan(u[:mw, :], dec[:mw, :], ph[:mw, :], 0.0, mult, add)
            nc.vector.tensor_scalar(
                out=s1b[:mw, m, 4:TW], in0=u[:mw, :],
                scalar1=THETA, scalar2=None, op0=is_ge,
            )
            if m == 1:
                _emit_l2(nc, mybir, b - 1, s1_tiles[b - 1], psum2, qpool,
                         opool, dec, w3t, sel_r, out_d)
        s1_tiles[b] = s1b

    _emit_l2(nc, mybir, n_batch - 1, s1_tiles[n_batch - 1], psum2, qpool,
             opool, dec, w3t, sel_r, out_d, tail=True)


def make_in_maps(spike_input, w1, d1, w3, d3):
    xpk, w1t, w3t, sel = _prep_host(spike_input, w1, d1, w3, d3)
    in_maps = []
    for c in range(N_CORES):
        in_maps.append({
            "xpk": np.ascontiguousarray(xpk[c * BPC:(c + 1) * BPC]),
            "w1t": w1t,
            "w3t": w3t,
            "sel": sel,
        })
    return in_maps


def kernel(spike_input, w1, d1, w3, d3):
    from concourse import bass_utils

    spike_input = np.asarray(spike_input, dtype=np.float32)
    w1 = np.asarray(w1, dtype=np.float32)
    d1 = np.asarray(d1, dtype=np.float32)
    w3 = np.asarray(w3, dtype=np.float32)
    d3 = np.asarray(d3, dtype=np.float32)

    nc = _build_nc()
    in_maps = make_in_maps(spike_input, w1, d1, w3, d3)
    res = bass_utils.run_bass_kernel_spmd(nc, in_maps, core_ids=list(range(N_CORES)))
    out = np.concatenate([res.results[c]["out"] for c in range(N_CORES)], axis=0)
    return out.astype(np.float32)
